# revision 18
# baseline (speedup 1.0000x reference)
"""Trainium2 Bass kernel for MinimalKAN forward (nn_MinimalKAN_Normalized).

Math:
  a = sigmoid(alpha)
  out = (1-a) * (x @ W.T + b) + (a/sqrt(I)) * (x @ C0 + x^2 @ C1 + x^3 @ C2)

Folding the alpha blend into the weights on the host gives exactly
  out = x @ A + x^2 @ B + x^3 @ C + b_eff
with A = (1-a) W.T + s C0, B = s C1, C = s C2, b_eff = (1-a) b, s = a/sqrt(I).

Device strategy (data-parallel over batch, 8 cores), per core 4096 rows.
The contraction index i sits on SBUF partitions; the host feeds x^T in fp16.
Mixed precision split by term magnitude:
  - linear term x @ A: fp16 matmuls (A host-scaled by S16=64 to clear the
    fp16 subnormal range), 4 accumulating matmuls per 128-row tile.
  - kan terms x^2 @ B + x^3 @ C: fp8(e4m3) matmuls in DoubleRow perf mode:
    lhsT [128, 2, 128] loads two k-planes (each PE cell holds 2 weights),
    K=256 per instruction at 1 col/cycle -> 2x the fp16 MAC rate.  B,C are
    tiny (~2e-4) so fp8 error lands well under tolerance; host-scaled by
    4096 to clear fp8 subnormals.  TRN e4m3 saturates at +-240: max|x|=5.4
    -> max|x^3| ~ 160, safe.
  - x^2 on ACT (Square, fp8 out), x^3 on GpSimd (x^2*x) at group
    granularity; PSUM merges + bias on DVE per tile; output stored bf16.
Per 128-row tile PE cost: 4*512 (fp16) + 4*512 (fp8 DR) = 4096 cycles vs
12*512 = 6144 all-fp16: ~57us PE floor at 2.3 GHz.
All HBM tensors are host-relayouted to [128 partitions, ...contiguous] so
every DMA kick is 128 descriptors of 1-4KB (descriptor-issue rate on the
two HWDGE rings otherwise limits the pipeline head/tail).  Kick sizing is
tuned for the pipeline head: weights ride the ACT ring in 128KB k-slices,
x^T rides the SP ring in 128KB tile slices, outputs drain per-tile on the
ACT ring.  A short burst of dummy matmuls during the initial fill starts
the PE p-state ramp early.
"""

import os
import numpy as np

import concourse.bass as bass
from concourse import bacc
import concourse.mybir as mybir
import concourse.tile as tile
from concourse.bass_utils import run_bass_kernel_spmd

N_CORES = 8
B, I, O = 32768, 512, 512
BS = B // N_CORES          # rows per core
P = 128
KS = I // P                # 4 contraction k-tiles per basis
N_TILES = BS // P          # 32 x 128-row tiles per core
G = int(os.environ.get("KAN_GROUP", "4"))     # tiles per x^2/x^3 group
N_GROUPS = N_TILES // G

S16 = 64.0                 # fp16 linear-weight host scale
S8A = 4096.0               # fp8 kan-weight host scale (x^2 and x^3 blocks)
N_WARM = int(os.environ.get("KAN_WARM", "6"))


def _build(repeat: int = 1) -> bass.Bass:
    f16 = mybir.dt.float16
    f8 = mybir.dt.float8e4
    f32 = mybir.dt.float32
    bf16 = mybir.dt.bfloat16
    sq = mybir.ActivationFunctionType.Square
    DR = mybir.MatmulPerfMode.DoubleRow
    mult = mybir.AluOpType.mult
    add = mybir.AluOpType.add

    nc = bacc.Bacc("TRN2", target_bir_lowering=False, debug=False,
                   num_devices=N_CORES)

    x_d = nc.dram_tensor("xt", [P, N_TILES, KS, P], f16,
                         kind="ExternalInput")
    wl_d = nc.dram_tensor("wlin", [P, KS, O], f16, kind="ExternalInput")
    wk_d = nc.dram_tensor("wkan", [P, 2 * KS, O], f8, kind="ExternalInput")
    b_d = nc.dram_tensor("bias", [P, O], f16, kind="ExternalInput")
    o_d = nc.dram_tensor("out", [P, N_TILES, O], bf16,
                         kind="ExternalOutput")

    with tile.TileContext(nc) as tc:
        with (
            tc.tile_pool(name="const", bufs=1) as const,
            tc.tile_pool(name="xin", bufs=3) as xin,
            tc.tile_pool(name="basis", bufs=3) as basis,
            tc.tile_pool(name="outp", bufs=8) as outp,
            tc.tile_pool(name="tmp", bufs=4) as tmpp,
            tc.tile_pool(name="ps_l", bufs=3, space="PSUM") as ps_l,
            tc.tile_pool(name="ps_k", bufs=3, space="PSUM") as ps_k,
            tc.tile_pool(name="ps_w", bufs=1, space="PSUM") as ps_w,
        ):
            # weights on the ACT ring, 128KB k-slices: the first linear
            # matmul only needs wl slice 0 + x^T tile 0.
            wl_sb = const.tile([P, KS, O], f16)
            for k in range(KS):
                nc.scalar.dma_start(wl_sb[:, k, :], wl_d[:, k, :])
            wk_sb = const.tile([P, 2 * KS, O], f8)
            for t in range(KS):
                nc.scalar.dma_start(wk_sb[:, 2 * t:2 * t + 2, :],
                                    wk_d[:, 2 * t:2 * t + 2, :])
            bsb = const.tile([P, O], f16)

            # PE p-state warmup during the initial fill (short 128-col
            # matmuls; results discarded).
            warm = const.tile([P, P], f16)
            nc.vector.memset(warm[:], 0.0)
            po_w = ps_w.tile([P, P], f32, tag="po_w")
            for _ in range(N_WARM):
                nc.tensor.matmul(po_w[:], warm[:], warm[:],
                                 start=True, stop=True,
                                 skip_group_check=True)

            for g in [i for _ in range(repeat) for i in range(N_GROUPS)]:
                xT = xin.tile([P, G, KS, P], f16, tag="xT")
                for j in range(G):
                    nc.sync.dma_start(xT[:, j], x_d[:, g * G + j])
                if g == 0:
                    nc.sync.dma_start(bsb[:], b_d[:])
                b8 = basis.tile([P, G, 2 * KS, P], f8, tag="b8")
                nc.scalar.activation(b8[:, :, 0:KS, :], xT[:], sq)
                nc.gpsimd.tensor_mul(b8[:, :, KS:2 * KS, :],
                                     b8[:, :, 0:KS, :], xT[:])
                for j in range(G):
                    po_l = ps_l.tile([P, O], f32, tag="po_l")
                    for k in range(KS):
                        nc.tensor.matmul(
                            po_l[:], xT[:, j, k, :], wl_sb[:, k, :],
                            start=(k == 0), stop=(k == KS - 1),
                            skip_group_check=True)
                    po_k = ps_k.tile([P, O], f32, tag="po_k")
                    for t in range(KS):
                        nc.tensor.matmul(
                            po_k[:],
                            b8[:, j, 2 * t:2 * t + 2, :],
                            wk_sb[:, 2 * t:2 * t + 2, :],
                            start=(t == 0), stop=(t == KS - 1),
                            perf_mode=DR, skip_group_check=True)
                    tmp = tmpp.tile([P, O], f32, tag="tmp")
                    nc.vector.scalar_tensor_tensor(
                        tmp[:], po_l[:], 1.0 / S16, bsb[:], mult, add)
                    o_t = outp.tile([P, O], bf16, tag="o_t")
                    nc.vector.scalar_tensor_tensor(
                        o_t[:], po_k[:], 1.0 / S8A, tmp[:], mult, add)
                    nc.scalar.dma_start(o_d[:, g * G + j, :], o_t[:])

    nc.compile()
    return nc


_NC_CACHE: dict[int, bass.Bass] = {}


def _get_nc(repeat: int = 1) -> bass.Bass:
    nc = _NC_CACHE.get(repeat)
    if nc is None:
        nc = _build(repeat)
        _NC_CACHE[repeat] = nc
    return nc


def _fold_weights(coeffs, W, b, alpha):
    a = 1.0 / (1.0 + np.exp(-np.float64(alpha)))
    s = a / np.sqrt(np.float64(I))
    A = (1.0 - a) * W.astype(np.float64).T + s * coeffs[:, :, 0].astype(np.float64)
    Bm = s * coeffs[:, :, 1].astype(np.float64)
    Cm = s * coeffs[:, :, 2].astype(np.float64)
    # [I, O] -> [P, KS, O] with row ks*P+p on partition p, slot ks
    wlin = (A * S16).astype(np.float16)
    wlin = np.ascontiguousarray(
        wlin.reshape(KS, P, O).transpose(1, 0, 2))
    f8np = mybir.dt.np(mybir.dt.float8e4)
    wkan = np.concatenate([Bm * S8A, Cm * S8A], axis=0)
    wkan = np.clip(wkan, -240.0, 240.0).astype(f8np)
    wkan = np.ascontiguousarray(
        wkan.reshape(2 * KS, P, O).transpose(1, 0, 2))
    b_eff = ((1.0 - a) * b.astype(np.float64)).astype(np.float16)
    bias_rep = np.ascontiguousarray(
        np.broadcast_to(b_eff[None, :], (P, O)))
    return wlin, wkan, bias_rep


def _make_in_maps(x, coeffs, W, b, alpha):
    wlin, wkan, bias_rep = _fold_weights(coeffs, W, b, alpha)
    x = np.asarray(x, dtype=np.float32)
    in_maps = []
    for c in range(N_CORES):
        shard = x[c * BS:(c + 1) * BS].astype(np.float16)
        # [BS, I] -> [P, N_TILES, KS, P]: xt[p, t, ks, c'] =
        # x[t*P+c', ks*P+p]
        xt = np.ascontiguousarray(
            shard.reshape(N_TILES, P, KS, P).transpose(3, 0, 2, 1))
        in_maps.append({
            "wlin": wlin, "wkan": wkan, "bias": bias_rep, "xt": xt,
        })
    return in_maps


def _unpack_out(raw):
    # [P, N_TILES, O] bf16 -> [BS, O] f32: row t*P + p
    return np.ascontiguousarray(
        np.asarray(raw).astype(np.float32).transpose(1, 0, 2)
    ).reshape(BS, O)


def _run(x, coeffs, W, b, alpha, trace=False):
    nc = _get_nc()
    in_maps = _make_in_maps(x, coeffs, W, b, alpha)
    res = run_bass_kernel_spmd(nc, in_maps, core_ids=list(range(N_CORES)),
                               trace=trace)
    out = np.concatenate([_unpack_out(r["out"]) for r in res.results], axis=0)
    return out, res


def kernel(x, coeffs, W, b, alpha):
    out, _ = _run(x, coeffs, W, b, alpha, trace=False)
    return out


# revision 20
# speedup vs baseline: 1.0306x; 1.0306x over previous
"""Trainium2 Bass kernel for MinimalKAN forward (nn_MinimalKAN_Normalized).

Math:
  a = sigmoid(alpha)
  out = (1-a) * (x @ W.T + b) + (a/sqrt(I)) * (x @ C0 + x^2 @ C1 + x^3 @ C2)

Folding the alpha blend into the weights on the host gives exactly
  out = x @ A + x^2 @ B + x^3 @ C + b_eff
with A = (1-a) W.T + s C0, B = s C1, C = s C2, b_eff = (1-a) b, s = a/sqrt(I).

Device strategy (data-parallel over batch, 8 cores), per core 4096 rows.
The contraction index i sits on SBUF partitions; the host feeds x^T in fp16.
Mixed precision split by term magnitude:
  - linear term x @ A: fp16 matmuls (A host-scaled by S16=64 to clear the
    fp16 subnormal range), 4 accumulating matmuls per 128-row tile.
  - kan terms x^2 @ B + x^3 @ C: fp8(e4m3) matmuls in DoubleRow perf mode:
    lhsT [128, 2, 128] loads two k-planes (each PE cell holds 2 weights),
    K=256 per instruction at 1 col/cycle -> 2x the fp16 MAC rate.  B,C are
    tiny (~2e-4) so fp8 error lands well under tolerance; host-scaled by
    4096 to clear fp8 subnormals.  TRN e4m3 saturates at +-240: max|x|=5.4
    -> max|x^3| ~ 160, safe.
  - x^2 on ACT (Square, fp8 out), x^3 on GpSimd (x^2*x) at group
    granularity; PSUM merges + bias on DVE per tile; output stored bf16.
Per 128-row tile PE cost: 4*512 (fp16) + 4*512 (fp8 DR) = 4096 cycles vs
12*512 = 6144 all-fp16: ~57us PE floor at 2.3 GHz.
All HBM tensors are host-relayouted to [128 partitions, ...contiguous] so
every DMA kick is 128 descriptors of 1-4KB (descriptor-issue rate on the
two HWDGE rings otherwise limits the pipeline head/tail).  Kick sizing is
tuned for the pipeline head: weights ride the ACT ring in 128KB k-slices,
x^T rides the SP ring in 128KB tile slices, outputs drain per-tile on the
ACT ring.  A short burst of dummy matmuls during the initial fill starts
the PE p-state ramp early.
"""

import os
import numpy as np

import concourse.bass as bass
from concourse import bacc
import concourse.mybir as mybir
import concourse.tile as tile
from concourse.bass_utils import run_bass_kernel_spmd

N_CORES = 8
B, I, O = 32768, 512, 512
BS = B // N_CORES          # rows per core
P = 128
KS = I // P                # 4 contraction k-tiles per basis
N_TILES = BS // P          # 32 x 128-row tiles per core
G = int(os.environ.get("KAN_GROUP", "4"))     # tiles per x^2/x^3 group
N_GROUPS = N_TILES // G

S16 = 64.0                 # fp16 linear-weight host scale
S8A = 4096.0               # fp8 kan-weight host scale (x^2 and x^3 blocks)
N_WARM = int(os.environ.get("KAN_WARM", "24"))


def _build(repeat: int = 1) -> bass.Bass:
    f16 = mybir.dt.float16
    f8 = mybir.dt.float8e4
    f32 = mybir.dt.float32
    bf16 = mybir.dt.bfloat16
    sq = mybir.ActivationFunctionType.Square
    DR = mybir.MatmulPerfMode.DoubleRow
    mult = mybir.AluOpType.mult
    add = mybir.AluOpType.add

    nc = bacc.Bacc("TRN2", target_bir_lowering=False, debug=False,
                   num_devices=N_CORES)

    x_d = nc.dram_tensor("xt", [P, N_TILES, KS, P], f16,
                         kind="ExternalInput")
    wl_d = nc.dram_tensor("wlin", [P, KS, O], f16, kind="ExternalInput")
    wk_d = nc.dram_tensor("wkan", [P, 2 * KS, O], f8, kind="ExternalInput")
    b_d = nc.dram_tensor("bias", [P, O], f16, kind="ExternalInput")
    o_d = nc.dram_tensor("out", [P, N_TILES, O], bf16,
                         kind="ExternalOutput")

    with tile.TileContext(nc) as tc:
        with (
            tc.tile_pool(name="const", bufs=1) as const,
            tc.tile_pool(name="xin", bufs=3) as xin,
            tc.tile_pool(name="basis", bufs=3) as basis,
            tc.tile_pool(name="outp", bufs=8) as outp,
            tc.tile_pool(name="tmp", bufs=4) as tmpp,
            tc.tile_pool(name="ps_l", bufs=3, space="PSUM") as ps_l,
            tc.tile_pool(name="ps_k", bufs=3, space="PSUM") as ps_k,
            tc.tile_pool(name="ps_w", bufs=1, space="PSUM") as ps_w,
        ):
            # weights on the ACT ring, 128KB k-slices: the first linear
            # matmul only needs wl slice 0 + x^T tile 0.
            wl_sb = const.tile([P, KS, O], f16)
            for k in range(KS):
                nc.scalar.dma_start(wl_sb[:, k, :], wl_d[:, k, :])
            wk_sb = const.tile([P, 2 * KS, O], f8)
            for t in range(KS):
                nc.scalar.dma_start(wk_sb[:, 2 * t:2 * t + 2, :],
                                    wk_d[:, 2 * t:2 * t + 2, :])
            bsb = const.tile([P, O], f16)

            # PE p-state warmup during the initial fill (short 128-col
            # matmuls; results discarded).
            warm = const.tile([P, P], f16)
            nc.vector.memset(warm[:], 0.0)
            po_w = ps_w.tile([P, P], f32, tag="po_w")
            for _ in range(N_WARM):
                nc.tensor.matmul(po_w[:], warm[:], warm[:],
                                 start=True, stop=True,
                                 skip_group_check=True)

            for g in [i for _ in range(repeat) for i in range(N_GROUPS)]:
                xT = xin.tile([P, G, KS, P], f16, tag="xT")
                for j in range(G):
                    nc.sync.dma_start(xT[:, j], x_d[:, g * G + j])
                if g == 0:
                    nc.sync.dma_start(bsb[:], b_d[:])
                b8 = basis.tile([P, G, 2 * KS, P], f8, tag="b8")
                if g == 0:
                    # first group: per-tile basis ops so the first kan
                    # matmuls don't wait on the whole group's x^T DMA
                    for j in range(G):
                        nc.scalar.activation(b8[:, j, 0:KS, :],
                                             xT[:, j], sq)
                        nc.gpsimd.tensor_mul(b8[:, j, KS:2 * KS, :],
                                             b8[:, j, 0:KS, :], xT[:, j])
                else:
                    nc.scalar.activation(b8[:, :, 0:KS, :], xT[:], sq)
                    nc.gpsimd.tensor_mul(b8[:, :, KS:2 * KS, :],
                                         b8[:, :, 0:KS, :], xT[:])
                for j in range(G):
                    po_l = ps_l.tile([P, O], f32, tag="po_l")
                    for k in range(KS):
                        nc.tensor.matmul(
                            po_l[:], xT[:, j, k, :], wl_sb[:, k, :],
                            start=(k == 0), stop=(k == KS - 1),
                            skip_group_check=True)
                    po_k = ps_k.tile([P, O], f32, tag="po_k")
                    for t in range(KS):
                        nc.tensor.matmul(
                            po_k[:],
                            b8[:, j, 2 * t:2 * t + 2, :],
                            wk_sb[:, 2 * t:2 * t + 2, :],
                            start=(t == 0), stop=(t == KS - 1),
                            perf_mode=DR, skip_group_check=True)
                    tmp = tmpp.tile([P, O], f32, tag="tmp")
                    nc.vector.scalar_tensor_tensor(
                        tmp[:], po_l[:], 1.0 / S16, bsb[:], mult, add)
                    o_t = outp.tile([P, O], bf16, tag="o_t")
                    nc.vector.scalar_tensor_tensor(
                        o_t[:], po_k[:], 1.0 / S8A, tmp[:], mult, add)
                    nc.scalar.dma_start(o_d[:, g * G + j, :], o_t[:])

    nc.compile()
    return nc


_NC_CACHE: dict[int, bass.Bass] = {}


def _get_nc(repeat: int = 1) -> bass.Bass:
    nc = _NC_CACHE.get(repeat)
    if nc is None:
        nc = _build(repeat)
        _NC_CACHE[repeat] = nc
    return nc


def _fold_weights(coeffs, W, b, alpha):
    a = 1.0 / (1.0 + np.exp(-np.float64(alpha)))
    s = a / np.sqrt(np.float64(I))
    A = (1.0 - a) * W.astype(np.float64).T + s * coeffs[:, :, 0].astype(np.float64)
    Bm = s * coeffs[:, :, 1].astype(np.float64)
    Cm = s * coeffs[:, :, 2].astype(np.float64)
    # [I, O] -> [P, KS, O] with row ks*P+p on partition p, slot ks
    wlin = (A * S16).astype(np.float16)
    wlin = np.ascontiguousarray(
        wlin.reshape(KS, P, O).transpose(1, 0, 2))
    f8np = mybir.dt.np(mybir.dt.float8e4)
    wkan = np.concatenate([Bm * S8A, Cm * S8A], axis=0)
    wkan = np.clip(wkan, -240.0, 240.0).astype(f8np)
    wkan = np.ascontiguousarray(
        wkan.reshape(2 * KS, P, O).transpose(1, 0, 2))
    b_eff = ((1.0 - a) * b.astype(np.float64)).astype(np.float16)
    bias_rep = np.ascontiguousarray(
        np.broadcast_to(b_eff[None, :], (P, O)))
    return wlin, wkan, bias_rep


def _make_in_maps(x, coeffs, W, b, alpha):
    wlin, wkan, bias_rep = _fold_weights(coeffs, W, b, alpha)
    x = np.asarray(x, dtype=np.float32)
    in_maps = []
    for c in range(N_CORES):
        shard = x[c * BS:(c + 1) * BS].astype(np.float16)
        # [BS, I] -> [P, N_TILES, KS, P]: xt[p, t, ks, c'] =
        # x[t*P+c', ks*P+p]
        xt = np.ascontiguousarray(
            shard.reshape(N_TILES, P, KS, P).transpose(3, 0, 2, 1))
        in_maps.append({
            "wlin": wlin, "wkan": wkan, "bias": bias_rep, "xt": xt,
        })
    return in_maps


def _unpack_out(raw):
    # [P, N_TILES, O] bf16 -> [BS, O] f32: row t*P + p
    return np.ascontiguousarray(
        np.asarray(raw).astype(np.float32).transpose(1, 0, 2)
    ).reshape(BS, O)


def _run(x, coeffs, W, b, alpha, trace=False):
    nc = _get_nc()
    in_maps = _make_in_maps(x, coeffs, W, b, alpha)
    res = run_bass_kernel_spmd(nc, in_maps, core_ids=list(range(N_CORES)),
                               trace=trace)
    out = np.concatenate([_unpack_out(r["out"]) for r in res.results], axis=0)
    return out, res


def kernel(x, coeffs, W, b, alpha):
    out, _ = _run(x, coeffs, W, b, alpha, trace=False)
    return out
